# revision 13
# baseline (speedup 1.0000x reference)
"""Trainium2 Bass kernel for nn_DenseAttention_85074712199779.

reference computation (B=8, N=8192, D=512, H=8, DH=64):
    x   = hs * cos + rotate_half(hs) * sin          # RoPE
    q   = x @ W.T                                   # dense projection
    kv  = einsum('bnhd,bnhe->bhde', xh, xh)         # per-head K^T V
    out = einsum('bnhd,bhde->bnhe', qh, kv)         # per-head Q (K^T V)

Sharding: batch-parallel -- core c owns batch c entirely (N=8192 rows),
so kv never crosses cores and there is NO collective.  All matmuls in
bf16 (fp32 PSUM accumulation); inputs pre-cast to bf16 on the host.

Per-core schedule (16 groups x 512 rows):
  - DMA hs + half-width cos/sin tables (cos[:, :256] == cos[:, 256:])
  - RoPE on DVE: x0 = hs0*c - hs1*s ; x1 = hs1*c + hs0*s
  - kv accumulation on PE (stationary = 128-wide head-pair slices)
  - x -> xT via the XBAR DMA-transpose (one instr per group), freeing
    the PE of 256 transpose matmuls and the DVE of their evacuations
  - qT = W @ xT on PE (stationary = W.T strips, 512-wide moving)
  - phase B: out chunk = qT.T @ blockdiag(kv head-pair), streamed DMA out
A few warm-up matmuls run at t=0 so the PE HAM clock-gate (1.2 GHz cold)
is released before real work arrives.
"""

import sys

if "/opt/trn_rl_repo" not in sys.path:
    sys.path.insert(0, "/opt/trn_rl_repo")

import numpy as np
import ml_dtypes

import concourse.bass as bass
import concourse.mybir as mybir
import concourse.tile as tile
from concourse import bacc

B = 8          # batch == cores
N = 8192       # sequence rows per core (whole batch)
D = 512        # hidden
NCORES = 8
G = 16         # groups per core
GC = 4         # chunks (128 rows) per group
BF16 = mybir.dt.bfloat16
F32 = mybir.dt.float32

_CACHE: dict = {}


def _build():
    nc = bacc.Bacc(trn_type="TRN2", num_devices=NCORES)

    # fused input rows: [hs(n) 512 | cos_half(n) 256 | sin_half(n) 256]
    # -- one DMA per group keeps the HWDGE queues nearly idle
    in_ext = nc.declare_dram_parameter("inrow", [N, 2 * D], BF16, isOutput=False)
    wt_ext = nc.declare_dram_parameter("wt", [D, D], BF16, isOutput=False)
    out_ext = nc.declare_dram_parameter("out", [N, D], BF16, isOutput=True)

    in_r = in_ext.rearrange("(g c p) d -> p g c d", p=128, c=GC)
    out_r = out_ext.rearrange("(u c p) d -> p u c d", p=128, c=2)

    with tile.TileContext(nc) as tc:
        with (
            tc.tile_pool(name="singles", bufs=1) as singles,
            tc.tile_pool(name="in_p", bufs=4) as in_p,
            tc.tile_pool(name="x_p", bufs=4) as x_p,
            tc.tile_pool(name="xt_p", bufs=4) as xt_p,
            tc.tile_pool(name="out_p", bufs=4) as out_p,
            tc.tile_pool(name="kv_ps", bufs=1, space="PSUM") as kv_ps,
            tc.tile_pool(name="q_ps", bufs=3, space="PSUM") as q_ps,
            tc.tile_pool(name="o_ps", bufs=3, space="PSUM") as o_ps,
        ):
            # ---- PE warm-up: release the HAM clock gate while the first
            # group's DMAs are in flight (junk matmuls on a zeroed tile) ----
            warm_sb = singles.tile([128, 128], BF16, name="warm_sb")
            nc.vector.memset(warm_sb, 0.0)
            warm_ps = o_ps.tile([128, 512], F32, name="op")
            for _ in range(24):
                nc.tensor.matmul(warm_ps[:, 0:128], warm_sb, warm_sb,
                                 start=True, stop=True)

            # DMA triggers run 3 groups ahead of compute and, on the scalar
            # queue, ahead of the qT evacuation copies emitted later in each
            # iteration -- a copy's semaphore wait must never block a trigger
            in_tiles = {}

            def emit_in_dma(g):
                in_t = in_p.tile([128, GC, 2 * D], BF16, name="in_t")
                nc.scalar.dma_start(out=in_t, in_=in_r[:, g])
                in_tiles[g] = in_t

            emit_in_dma(0)

            # W.T strips: wt_sb[p, db, e] = W[e, db*128+p]
            wt_sb = singles.tile([128, 4, D], BF16, name="wt_sb")
            nc.scalar.dma_start(out=wt_sb,
                                in_=wt_ext.rearrange("(b p) e -> p b e", p=128))

            kvblk = singles.tile([128, 4, 128], BF16, name="kvblk")
            nc.gpsimd.memset(kvblk, 0.0)

            qT_sb = singles.tile([128, 4, N], BF16, name="qT_sb")
            kvp = kv_ps.tile([128, 4, 128], F32, name="kvp")

            xt_tiles = {}
            x_tiles = {}

            def emit_qproj(g):
                xt_t = xt_tiles.pop(g)
                for eb in range(4):
                    qp = q_ps.tile([128, 512], F32, name="qp")
                    for db in range(4):
                        nc.tensor.matmul(
                            qp, wt_sb[:, db, eb * 128:(eb + 1) * 128],
                            xt_t[:, :, db, :],
                            start=(db == 0), stop=(db == 3))
                    nc.scalar.copy(
                        out=qT_sb[:, eb, g * 512:(g + 1) * 512], in_=qp)

            def emit_kv(g):
                x_t = x_tiles.pop(g)
                for c in range(GC):
                    for hp in range(4):
                        xs = x_t[:, c, hp * 128:(hp + 1) * 128]
                        nc.tensor.matmul(
                            kvp[:, hp, :], xs, xs,
                            start=(g == 0 and c == 0 and hp == 0),
                            stop=(g == G - 1 and c == GC - 1 and hp == 3))

            emit_in_dma(1)
            emit_in_dma(2)

            for g in range(G):
                if g + 3 < G:
                    emit_in_dma(g + 3)
                in_t = in_tiles.pop(g)

                # RoPE (DVE): x0 = hs0*c - hs1*s ; x1 = hs1*c + hs0*s
                x_t = x_p.tile([128, GC, D], BF16, name="x_t")
                hs0 = in_t[:, :, 0:256]
                hs1 = in_t[:, :, 256:512]
                c_t = in_t[:, :, 512:768]
                s_t = in_t[:, :, 768:1024]
                x0 = x_t[:, :, 0:256]
                x1 = x_t[:, :, 256:512]
                mul = mybir.AluOpType.mult
                nc.vector.tensor_tensor(x0, hs1, s_t, mul)
                nc.vector.tensor_tensor(x1, hs0, s_t, mul)
                nc.vector.tensor_tensor(hs0, hs0, c_t, mul)
                nc.vector.tensor_tensor(hs1, hs1, c_t, mul)
                nc.vector.tensor_tensor(x0, hs0, x0, mybir.AluOpType.subtract)
                nc.vector.tensor_tensor(x1, hs1, x1, mybir.AluOpType.add)

                # x -> xT via XBAR DMA-transpose (one instruction):
                # xt_t[d, c*4+db, n] = x_t[n, c, db*128+d]
                xt_t = xt_p.tile([128, GC, 4, 128], BF16, name="xt_t")
                xt_tiles[g] = xt_t
                x_tiles[g] = x_t
                nc.sync.dma_start(
                    out=xt_t.rearrange("p c b n -> p (c b) n"),
                    in_=x_t, transpose=True)

                # PE work lags so cross-engine semaphore latency stays
                # hidden: qproj two groups back, kv one group back
                if g >= 2:
                    emit_qproj(g - 2)
                if g >= 1:
                    emit_kv(g - 1)

            emit_qproj(G - 2)
            emit_kv(G - 1)

            # kv -> block-diagonal bf16 (per-head 64x64 diag blocks);
            # qproj(G-1) covers the copy + semaphore latency on the PE
            nc.any.tensor_copy(out=kvblk[0:64, :, 0:64], in_=kvp[0:64, :, 0:64])
            nc.any.tensor_copy(out=kvblk[64:128, :, 64:128],
                               in_=kvp[64:128, :, 64:128])

            emit_qproj(G - 1)

            # ---------------- phase B ----------------
            for u in range(N // 256):
                out_sb = out_p.tile([128, 2, D], BF16, name="out_sb")
                for cc in range(2):
                    ci = u * 2 + cc
                    op = o_ps.tile([128, 512], F32, name="op")
                    for hp in range(4):
                        nc.tensor.matmul(
                            op[:, hp * 128:(hp + 1) * 128],
                            qT_sb[:, hp, ci * 128:(ci + 1) * 128],
                            kvblk[:, hp, :],
                            start=True, stop=True)
                    if (u * 2 + cc) % 2 == 0:
                        nc.vector.tensor_copy(out=out_sb[:, cc, :], in_=op)
                    else:
                        nc.scalar.copy(out=out_sb[:, cc, :], in_=op)
                nc.sync.dma_start(out=out_r[:, u], in_=out_sb)

    nc.compile()
    return nc


def _prep_in_maps(hidden_states, W, cos, sin):
    bf16 = ml_dtypes.bfloat16
    hs = np.asarray(hidden_states, dtype=np.float32)
    cos_h = np.asarray(cos, dtype=np.float32)[:, : D // 2]
    sin_h = np.asarray(sin, dtype=np.float32)[:, : D // 2]
    wt16 = np.ascontiguousarray(np.asarray(W, dtype=np.float32).T).astype(bf16)
    in_maps = []
    for c in range(NCORES):
        row = np.concatenate([hs[c], cos_h, sin_h], axis=1)
        in_maps.append({
            "inrow": np.ascontiguousarray(row).astype(bf16),
            "wt": wt16,
        })
    return in_maps


def _collect(results):
    out = np.empty((B, N, D), dtype=np.float32)
    for c in range(NCORES):
        out[c] = results[c]["out"].astype(np.float32)
    return out


def kernel(hidden_states, W, cos, sin):
    from concourse.bass_utils import run_bass_kernel_spmd

    nc = _CACHE.get("nc")
    if nc is None:
        nc = _build()
        _CACHE["nc"] = nc

    in_maps = _prep_in_maps(hidden_states, W, cos, sin)
    res = run_bass_kernel_spmd(nc, in_maps, list(range(NCORES)))
    return _collect(res.results)


# revision 15
# speedup vs baseline: 1.0630x; 1.0630x over previous
"""Trainium2 Bass kernel for nn_DenseAttention_85074712199779.

reference computation (B=8, N=8192, D=512, H=8, DH=64):
    x   = hs * cos + rotate_half(hs) * sin          # RoPE
    q   = x @ W.T                                   # dense projection
    kv  = einsum('bnhd,bnhe->bhde', xh, xh)         # per-head K^T V
    out = einsum('bnhd,bhde->bnhe', qh, kv)         # per-head Q (K^T V)

Sharding: batch-parallel -- core c owns batch c entirely (N=8192 rows),
so kv never crosses cores and there is NO collective.  All matmuls in
bf16 (fp32 PSUM accumulation); inputs pre-cast to bf16 on the host.

Per-core schedule (16 groups x 512 rows):
  - DMA hs + half-width cos/sin tables (cos[:, :256] == cos[:, 256:])
  - RoPE on DVE: x0 = hs0*c - hs1*s ; x1 = hs1*c + hs0*s
  - kv accumulation on PE (stationary = 128-wide head-pair slices)
  - x -> xT via the XBAR DMA-transpose (one instr per group), freeing
    the PE of 256 transpose matmuls and the DVE of their evacuations
  - qT = W @ xT on PE (stationary = W.T strips, 512-wide moving)
  - phase B: out chunk = qT.T @ blockdiag(kv head-pair), streamed DMA out
A few warm-up matmuls run at t=0 so the PE HAM clock-gate (1.2 GHz cold)
is released before real work arrives.
"""

import sys

if "/opt/trn_rl_repo" not in sys.path:
    sys.path.insert(0, "/opt/trn_rl_repo")

import numpy as np
import ml_dtypes

import concourse.bass as bass
import concourse.mybir as mybir
import concourse.tile as tile
from concourse import bacc

B = 8          # batch == cores
N = 8192       # sequence rows per core (whole batch)
D = 512        # hidden
NCORES = 8
G = 16         # groups per core
GC = 4         # chunks (128 rows) per group
BF16 = mybir.dt.bfloat16
F32 = mybir.dt.float32

_CACHE: dict = {}


def _build():
    nc = bacc.Bacc(trn_type="TRN2", num_devices=NCORES)

    # fused input rows: [hs(n) 512 | cos_half(n) 256 | sin_half(n) 256]
    # -- one DMA per group keeps the HWDGE queues nearly idle
    in_ext = nc.declare_dram_parameter("inrow", [N, 2 * D], BF16, isOutput=False)
    wt_ext = nc.declare_dram_parameter("wt", [D, D], BF16, isOutput=False)
    out_ext = nc.declare_dram_parameter("out", [N, D], BF16, isOutput=True)

    in_r = in_ext.rearrange("(g c p) d -> p g c d", p=128, c=GC)
    out_r = out_ext.rearrange("(u c p) d -> p u c d", p=128, c=2)

    with tile.TileContext(nc) as tc:
        with (
            tc.tile_pool(name="singles", bufs=1) as singles,
            tc.tile_pool(name="in_p", bufs=4) as in_p,
            tc.tile_pool(name="x_p", bufs=4) as x_p,
            tc.tile_pool(name="xt_p", bufs=4) as xt_p,
            tc.tile_pool(name="out_p", bufs=4) as out_p,
            tc.tile_pool(name="kv_ps", bufs=1, space="PSUM") as kv_ps,
            tc.tile_pool(name="q_ps", bufs=3, space="PSUM") as q_ps,
            tc.tile_pool(name="o_ps", bufs=3, space="PSUM") as o_ps,
        ):
            # ---- PE warm-up: release the HAM clock gate while the first
            # group's DMAs are in flight (junk matmuls on a zeroed tile) ----
            warm_sb = singles.tile([128, 128], BF16, name="warm_sb")
            nc.vector.memset(warm_sb, 0.0)
            warm_ps = o_ps.tile([128, 512], F32, name="op")
            for _ in range(24):
                nc.tensor.matmul(warm_ps[:, 0:128], warm_sb, warm_sb,
                                 start=True, stop=True)

            # DMA triggers run 3 groups ahead of compute and, on the scalar
            # queue, ahead of the qT evacuation copies emitted later in each
            # iteration -- a copy's semaphore wait must never block a trigger
            in_tiles = {}

            def emit_in_dma(g):
                # split across both HWDGE queues: per-queue DMA throughput
                # (~145 GB/s) would otherwise serialize the pipeline
                in_t = in_p.tile([128, GC, 2 * D], BF16, name="in_t")
                nc.sync.dma_start(out=in_t[:, 0:2, :], in_=in_r[:, g, 0:2, :])
                nc.scalar.dma_start(out=in_t[:, 2:4, :], in_=in_r[:, g, 2:4, :])
                in_tiles[g] = in_t

            emit_in_dma(0)

            # W.T strips: wt_sb[p, db, e] = W[e, db*128+p]
            wt_sb = singles.tile([128, 4, D], BF16, name="wt_sb")
            nc.scalar.dma_start(out=wt_sb,
                                in_=wt_ext.rearrange("(b p) e -> p b e", p=128))

            kvblk = singles.tile([128, 4, 128], BF16, name="kvblk")
            nc.gpsimd.memset(kvblk, 0.0)

            qT_sb = singles.tile([128, 4, N], BF16, name="qT_sb")
            kvp = kv_ps.tile([128, 4, 128], F32, name="kvp")

            xt_tiles = {}
            x_tiles = {}

            def emit_qproj(g):
                xt_t = xt_tiles.pop(g)
                for eb in range(4):
                    qp = q_ps.tile([128, 512], F32, name="qp")
                    for db in range(4):
                        nc.tensor.matmul(
                            qp, wt_sb[:, db, eb * 128:(eb + 1) * 128],
                            xt_t[:, :, db, :],
                            start=(db == 0), stop=(db == 3))
                    nc.scalar.copy(
                        out=qT_sb[:, eb, g * 512:(g + 1) * 512], in_=qp)

            def emit_kv(g):
                x_t = x_tiles.pop(g)
                for c in range(GC):
                    for hp in range(4):
                        xs = x_t[:, c, hp * 128:(hp + 1) * 128]
                        nc.tensor.matmul(
                            kvp[:, hp, :], xs, xs,
                            start=(g == 0 and c == 0 and hp == 0),
                            stop=(g == G - 1 and c == GC - 1 and hp == 3))

            emit_in_dma(1)
            emit_in_dma(2)

            for g in range(G):
                if g + 3 < G:
                    emit_in_dma(g + 3)
                in_t = in_tiles.pop(g)

                # RoPE (DVE): x0 = hs0*c - hs1*s ; x1 = hs1*c + hs0*s
                x_t = x_p.tile([128, GC, D], BF16, name="x_t")
                hs0 = in_t[:, :, 0:256]
                hs1 = in_t[:, :, 256:512]
                c_t = in_t[:, :, 512:768]
                s_t = in_t[:, :, 768:1024]
                x0 = x_t[:, :, 0:256]
                x1 = x_t[:, :, 256:512]
                mul = mybir.AluOpType.mult
                nc.vector.tensor_tensor(x0, hs1, s_t, mul)
                nc.vector.tensor_tensor(x1, hs0, s_t, mul)
                nc.vector.tensor_tensor(hs0, hs0, c_t, mul)
                nc.vector.tensor_tensor(hs1, hs1, c_t, mul)
                nc.vector.tensor_tensor(x0, hs0, x0, mybir.AluOpType.subtract)
                nc.vector.tensor_tensor(x1, hs1, x1, mybir.AluOpType.add)

                # x -> xT via XBAR DMA-transpose (one instruction):
                # xt_t[d, c*4+db, n] = x_t[n, c, db*128+d]
                xt_t = xt_p.tile([128, GC, 4, 128], BF16, name="xt_t")
                xt_tiles[g] = xt_t
                x_tiles[g] = x_t
                nc.sync.dma_start(
                    out=xt_t.rearrange("p c b n -> p (c b) n"),
                    in_=x_t, transpose=True)

                # PE work lags so cross-engine semaphore latency stays
                # hidden: qproj two groups back, kv one group back
                if g >= 2:
                    emit_qproj(g - 2)
                if g >= 1:
                    emit_kv(g - 1)

            emit_qproj(G - 2)
            emit_kv(G - 1)

            # kv -> block-diagonal bf16 (per-head 64x64 diag blocks);
            # qproj(G-1) covers the copy + semaphore latency on the PE
            nc.any.tensor_copy(out=kvblk[0:64, :, 0:64], in_=kvp[0:64, :, 0:64])
            nc.any.tensor_copy(out=kvblk[64:128, :, 64:128],
                               in_=kvp[64:128, :, 64:128])

            emit_qproj(G - 1)

            # ---------------- phase B ----------------
            for u in range(N // 256):
                out_sb = out_p.tile([128, 2, D], BF16, name="out_sb")
                for cc in range(2):
                    ci = u * 2 + cc
                    op = o_ps.tile([128, 512], F32, name="op")
                    for hp in range(4):
                        nc.tensor.matmul(
                            op[:, hp * 128:(hp + 1) * 128],
                            qT_sb[:, hp, ci * 128:(ci + 1) * 128],
                            kvblk[:, hp, :],
                            start=True, stop=True)
                    if (u * 2 + cc) % 2 == 0:
                        nc.vector.tensor_copy(out=out_sb[:, cc, :], in_=op)
                    else:
                        nc.scalar.copy(out=out_sb[:, cc, :], in_=op)
                oq = nc.sync if u % 2 == 0 else nc.scalar
                oq.dma_start(out=out_r[:, u], in_=out_sb)

    nc.compile()
    return nc


def _prep_in_maps(hidden_states, W, cos, sin):
    bf16 = ml_dtypes.bfloat16
    hs = np.asarray(hidden_states, dtype=np.float32)
    cos_h = np.asarray(cos, dtype=np.float32)[:, : D // 2]
    sin_h = np.asarray(sin, dtype=np.float32)[:, : D // 2]
    wt16 = np.ascontiguousarray(np.asarray(W, dtype=np.float32).T).astype(bf16)
    in_maps = []
    for c in range(NCORES):
        row = np.concatenate([hs[c], cos_h, sin_h], axis=1)
        in_maps.append({
            "inrow": np.ascontiguousarray(row).astype(bf16),
            "wt": wt16,
        })
    return in_maps


def _collect(results):
    out = np.empty((B, N, D), dtype=np.float32)
    for c in range(NCORES):
        out[c] = results[c]["out"].astype(np.float32)
    return out


def kernel(hidden_states, W, cos, sin):
    from concourse.bass_utils import run_bass_kernel_spmd

    nc = _CACHE.get("nc")
    if nc is None:
        nc = _build()
        _CACHE["nc"] = nc

    in_maps = _prep_in_maps(hidden_states, W, cos, sin)
    res = run_bass_kernel_spmd(nc, in_maps, list(range(NCORES)))
    return _collect(res.results)


# revision 16
# speedup vs baseline: 1.0982x; 1.0332x over previous
"""Trainium2 Bass kernel for nn_DenseAttention_85074712199779.

reference computation (B=8, N=8192, D=512, H=8, DH=64):
    x   = hs * cos + rotate_half(hs) * sin          # RoPE
    q   = x @ W.T                                   # dense projection
    kv  = einsum('bnhd,bnhe->bhde', xh, xh)         # per-head K^T V
    out = einsum('bnhd,bhde->bnhe', qh, kv)         # per-head Q (K^T V)

Sharding: batch-parallel -- core c owns batch c entirely (N=8192 rows),
so kv never crosses cores and there is NO collective.  All matmuls in
bf16 (fp32 PSUM accumulation); inputs pre-cast to bf16 on the host.

Per-core schedule (16 groups x 512 rows):
  - DMA hs + half-width cos/sin tables (cos[:, :256] == cos[:, 256:])
  - RoPE on DVE: x0 = hs0*c - hs1*s ; x1 = hs1*c + hs0*s
  - kv accumulation on PE (stationary = 128-wide head-pair slices)
  - x -> xT via the XBAR DMA-transpose (one instr per group), freeing
    the PE of 256 transpose matmuls and the DVE of their evacuations
  - qT = W @ xT on PE (stationary = W.T strips, 512-wide moving)
  - phase B: out chunk = qT.T @ blockdiag(kv head-pair), streamed DMA out
A few warm-up matmuls run at t=0 so the PE HAM clock-gate (1.2 GHz cold)
is released before real work arrives.
"""

import sys

if "/opt/trn_rl_repo" not in sys.path:
    sys.path.insert(0, "/opt/trn_rl_repo")

import numpy as np
import ml_dtypes

import concourse.bass as bass
import concourse.mybir as mybir
import concourse.tile as tile
from concourse import bacc

B = 8          # batch == cores
N = 8192       # sequence rows per core (whole batch)
D = 512        # hidden
NCORES = 8
G = 16         # groups per core
GC = 4         # chunks (128 rows) per group
BF16 = mybir.dt.bfloat16
F32 = mybir.dt.float32

_CACHE: dict = {}


def _build():
    nc = bacc.Bacc(trn_type="TRN2", num_devices=NCORES)

    # fused input rows: [hs(n) 512 | cos_half(n) 256 | sin_half(n) 256]
    # -- one DMA per group keeps the HWDGE queues nearly idle
    in_ext = nc.declare_dram_parameter("inrow", [N, 2 * D], BF16, isOutput=False)
    wt_ext = nc.declare_dram_parameter("wt", [D, D], BF16, isOutput=False)
    out_ext = nc.declare_dram_parameter("out", [N, D], BF16, isOutput=True)

    in_r = in_ext.rearrange("(g c p) d -> p g c d", p=128, c=GC)
    out_r = out_ext.rearrange("(u c p) d -> p u c d", p=128, c=2)

    with tile.TileContext(nc) as tc:
        with (
            tc.tile_pool(name="singles", bufs=1) as singles,
            tc.tile_pool(name="in_p", bufs=4) as in_p,
            tc.tile_pool(name="x_p", bufs=4) as x_p,
            tc.tile_pool(name="xt_p", bufs=4) as xt_p,
            tc.tile_pool(name="out_p", bufs=4) as out_p,
            tc.tile_pool(name="kv_ps", bufs=1, space="PSUM") as kv_ps,
            tc.tile_pool(name="q_ps", bufs=3, space="PSUM") as q_ps,
            tc.tile_pool(name="o_ps", bufs=3, space="PSUM") as o_ps,
        ):
            # ---- PE warm-up: release the HAM clock gate while the first
            # group's DMAs are in flight (junk matmuls on a zeroed tile) ----
            warm_sb = singles.tile([128, 128], BF16, name="warm_sb")
            nc.vector.memset(warm_sb, 0.0)
            warm_ps = o_ps.tile([128, 512], F32, name="op")
            for _ in range(24):
                nc.tensor.matmul(warm_ps[:, 0:128], warm_sb, warm_sb,
                                 start=True, stop=True)

            # DMA triggers run 3 groups ahead of compute and, on the scalar
            # queue, ahead of the qT evacuation copies emitted later in each
            # iteration -- a copy's semaphore wait must never block a trigger
            in_tiles = {}

            def emit_in_dma(g):
                # split across both HWDGE queues: per-queue DMA throughput
                # (~145 GB/s) would otherwise serialize the pipeline
                in_t = in_p.tile([128, GC, 2 * D], BF16, name="in_t")
                nc.sync.dma_start(out=in_t[:, 0:2, :], in_=in_r[:, g, 0:2, :])
                nc.scalar.dma_start(out=in_t[:, 2:4, :], in_=in_r[:, g, 2:4, :])
                in_tiles[g] = in_t

            emit_in_dma(0)

            # W.T strips: wt_sb[p, db, e] = W[e, db*128+p]
            wt_sb = singles.tile([128, 4, D], BF16, name="wt_sb")
            nc.scalar.dma_start(out=wt_sb,
                                in_=wt_ext.rearrange("(b p) e -> p b e", p=128))

            kvblk = singles.tile([128, 4, 128], BF16, name="kvblk")
            nc.gpsimd.memset(kvblk, 0.0)

            qT_sb = singles.tile([128, 4, N], BF16, name="qT_sb")
            kvp = kv_ps.tile([128, 4, 128], F32, name="kvp")

            xt_tiles = {}
            x_tiles = {}

            def emit_qproj(g):
                xt_t = xt_tiles.pop(g)
                for eb in range(4):
                    qp = q_ps.tile([128, 512], F32, name="qp")
                    for db in range(4):
                        nc.tensor.matmul(
                            qp, wt_sb[:, db, eb * 128:(eb + 1) * 128],
                            xt_t[:, :, db, :],
                            start=(db == 0), stop=(db == 3))
                    nc.scalar.copy(
                        out=qT_sb[:, eb, g * 512:(g + 1) * 512], in_=qp)

            def emit_xbar(g):
                # x -> xT via XBAR DMA-transpose (one instruction):
                # xt_t[d, c*4+db, n] = x_t[n, c, db*128+d]
                xt_t = xt_p.tile([128, GC, 4, 128], BF16, name="xt_t")
                xt_tiles[g] = xt_t
                nc.sync.dma_start(
                    out=xt_t.rearrange("p c b n -> p (c b) n"),
                    in_=x_tiles[g], transpose=True)

            def emit_kv(g):
                x_t = x_tiles.pop(g)
                for c in range(GC):
                    for hp in range(4):
                        xs = x_t[:, c, hp * 128:(hp + 1) * 128]
                        nc.tensor.matmul(
                            kvp[:, hp, :], xs, xs,
                            start=(g == 0 and c == 0 and hp == 0),
                            stop=(g == G - 1 and c == GC - 1 and hp == 3))

            emit_in_dma(1)
            emit_in_dma(2)

            for g in range(G):
                if g + 3 < G:
                    emit_in_dma(g + 3)
                in_t = in_tiles.pop(g)

                # RoPE (DVE): x0 = hs0*c - hs1*s ; x1 = hs1*c + hs0*s
                x_t = x_p.tile([128, GC, D], BF16, name="x_t")
                hs0 = in_t[:, :, 0:256]
                hs1 = in_t[:, :, 256:512]
                c_t = in_t[:, :, 512:768]
                s_t = in_t[:, :, 768:1024]
                x0 = x_t[:, :, 0:256]
                x1 = x_t[:, :, 256:512]
                mul = mybir.AluOpType.mult
                nc.vector.tensor_tensor(x0, hs1, s_t, mul)
                nc.vector.tensor_tensor(x1, hs0, s_t, mul)
                nc.vector.tensor_tensor(hs0, hs0, c_t, mul)
                nc.vector.tensor_tensor(hs1, hs1, c_t, mul)
                nc.vector.tensor_tensor(x0, hs0, x0, mybir.AluOpType.subtract)
                nc.vector.tensor_tensor(x1, hs1, x1, mybir.AluOpType.add)

                x_tiles[g] = x_t

                # XBAR transpose one iteration late: by the time the sync
                # queue reaches it, its RoPE dependency is already done, so
                # the in-order queue never idle-waits
                if g >= 1:
                    emit_xbar(g - 1)
                if g >= 2:
                    emit_qproj(g - 2)
                if g >= 1:
                    emit_kv(g - 1)

            emit_xbar(G - 1)
            emit_qproj(G - 2)
            emit_kv(G - 1)

            # kv -> block-diagonal bf16 (per-head 64x64 diag blocks);
            # qproj(G-1) covers the copy + semaphore latency on the PE
            nc.any.tensor_copy(out=kvblk[0:64, :, 0:64], in_=kvp[0:64, :, 0:64])
            nc.any.tensor_copy(out=kvblk[64:128, :, 64:128],
                               in_=kvp[64:128, :, 64:128])

            emit_qproj(G - 1)

            # ---------------- phase B ----------------
            for u in range(N // 256):
                out_sb = out_p.tile([128, 2, D], BF16, name="out_sb")
                for cc in range(2):
                    ci = u * 2 + cc
                    op = o_ps.tile([128, 512], F32, name="op")
                    for hp in range(4):
                        nc.tensor.matmul(
                            op[:, hp * 128:(hp + 1) * 128],
                            qT_sb[:, hp, ci * 128:(ci + 1) * 128],
                            kvblk[:, hp, :],
                            start=True, stop=True)
                    if (u * 2 + cc) % 2 == 0:
                        nc.vector.tensor_copy(out=out_sb[:, cc, :], in_=op)
                    else:
                        nc.scalar.copy(out=out_sb[:, cc, :], in_=op)
                oq = nc.sync if u % 2 == 0 else nc.scalar
                oq.dma_start(out=out_r[:, u], in_=out_sb)

    nc.compile()
    return nc


def _prep_in_maps(hidden_states, W, cos, sin):
    bf16 = ml_dtypes.bfloat16
    hs = np.asarray(hidden_states, dtype=np.float32)
    cos_h = np.asarray(cos, dtype=np.float32)[:, : D // 2]
    sin_h = np.asarray(sin, dtype=np.float32)[:, : D // 2]
    wt16 = np.ascontiguousarray(np.asarray(W, dtype=np.float32).T).astype(bf16)
    in_maps = []
    for c in range(NCORES):
        row = np.concatenate([hs[c], cos_h, sin_h], axis=1)
        in_maps.append({
            "inrow": np.ascontiguousarray(row).astype(bf16),
            "wt": wt16,
        })
    return in_maps


def _collect(results):
    out = np.empty((B, N, D), dtype=np.float32)
    for c in range(NCORES):
        out[c] = results[c]["out"].astype(np.float32)
    return out


def kernel(hidden_states, W, cos, sin):
    from concourse.bass_utils import run_bass_kernel_spmd

    nc = _CACHE.get("nc")
    if nc is None:
        nc = _build()
        _CACHE["nc"] = nc

    in_maps = _prep_in_maps(hidden_states, W, cos, sin)
    res = run_bass_kernel_spmd(nc, in_maps, list(range(NCORES)))
    return _collect(res.results)
